# revision 18
# baseline (speedup 1.0000x reference)
"""Trainium2 Bass kernel for nn_NodeModel (GNN message passing).

Reference computation:
    agg = segment_sum(edge_attr, edge_index[1], num_segments=N)     # scatter-add
    h   = relu(concat([x, agg, u[batch]], 1) @ W1 + b1)
    out = h @ W2 + b2 + x

Strategy (8 NeuronCores, graph-parallel by destination node):
  - Nodes are padded to 100352 = 8 * 12544 and sharded contiguously across 8
    cores. Each core owns 12544 destination nodes = 98 ranges of 128 nodes.
  - Host groups edges by destination range (counting-sort), pads each range's
    edge list to a multiple of 128, and ships per-core edge features in bf16
    (the scatter matmul consumes bf16; host-side cast halves the HBM stream).
  - Scatter-add per 128-edge block is one TensorE matmul:
    PSUM[feat, node] += ea_blk(bf16, stationary).T @ onehot_blk(moving).
    One-hot construction is split between DVE (tensor_scalar is_equal against
    an iota row, ~153 ns/block measured) and host-built fp8 blocks shipped
    over DMA (16 KB/block) — the split ratio balances DVE time against the
    HBM bottleneck. Host chunks are spread evenly through the schedule.
  - MLP hidden runs per 512-node group transposed: h[hid, n] (PSUM) =
    W1x.T xT + W1a.T aggT + W1u.T ugT; ReLU+bias fused into ScalarE PSUM
    evacuation; hs stored bf16.
  - Layer 2 runs per 128-node tile in NATURAL orientation (no transposes):
    out[n, d] (PSUM) = hs0_t.T W2a + hs1_t.T W2b + I.T (x + b2)_t, the last
    matmul folding residual AND bias via host-precomputed xbn = x + b2.
  - xbn and out use a per-group tile layout in DRAM so their DMAs are fully
    contiguous (host un-swizzles the output; out ships bf16, cast on host).
"""

import os
from contextlib import ExitStack

import ml_dtypes
import numpy as np

N_NODES = 100000
N_EDGES = 1600000
D = 128          # node / edge feature dim
DG = 16          # global feature dim
H = 256          # hidden dim
NCORES = 8

NPC = 12544      # nodes per core (= 98 * 128)
N_PAD = NCORES * NPC
RW = 128         # scatter range width (nodes per PSUM accumulation group)
RPC = NPC // RW  # 98 ranges per core
EBLK = 128       # edges per matmul block
CHUNK_BLKS = 64  # edge blocks per DMA chunk (8192 edges: 2 MiB ea + 1 MiB oh)
HOST_OH_BLKS = int(os.environ.get("HOST_OH_BLKS", "26"))  # of CHUNK_BLKS per chunk

NB_MLP = 512     # nodes per MLP group
NGRP = (NPC + NB_MLP - 1) // NB_MLP

_PROFILE_RESULTS = [None]  # stash for test harness introspection


def _host_blk_flags():
    """Evenly-spread boolean pattern: which of a chunk's 64 blocks ship a
    host-built fp8 one-hot (the rest are built on DVE)."""
    k = HOST_OH_BLKS
    flags = [(co * k) // CHUNK_BLKS != ((co + 1) * k) // CHUNK_BLKS
             for co in range(CHUNK_BLKS)]
    return flags


def _shard_inputs(x, edge_index, edge_attr, u, batch, W1, b1, W2, b2):
    bf16 = ml_dtypes.bfloat16
    fp8 = ml_dtypes.float8_e4m3
    x = np.ascontiguousarray(np.asarray(x, dtype=np.float32))
    edge_attr = np.ascontiguousarray(np.asarray(edge_attr, dtype=np.float32))
    u = np.asarray(u, dtype=np.float32)
    batch = np.asarray(batch)
    W1 = np.asarray(W1, dtype=np.float32)
    b1 = np.asarray(b1, dtype=np.float32)
    W2 = np.asarray(W2, dtype=np.float32)
    b2 = np.asarray(b2, dtype=np.float32)

    col = np.asarray(edge_index[1], dtype=np.int64)
    r_glob = col // RW                               # global range id
    n_ranges = NCORES * RPC

    counts = np.bincount(r_glob, minlength=n_ranges)
    cnt_cl = counts.reshape(NCORES, RPC)
    # blocks per local range: shared across cores (same SPMD program)
    B = np.maximum(1, (cnt_cl.max(axis=0) + EBLK - 1) // EBLK).astype(np.int64)
    prefix = np.concatenate([[0], np.cumsum(B)])    # [RPC+1]
    nblk = int(prefix[-1])                          # blocks per core
    s_slots = nblk * EBLK
    nchunk = (s_slots + CHUNK_BLKS * EBLK - 1) // (CHUNK_BLKS * EBLK)
    s_alloc = nchunk * CHUNK_BLKS * EBLK
    nblk_alloc = s_alloc // EBLK

    order = np.argsort(r_glob, kind="stable")
    sorted_r = r_glob[order]
    starts = np.concatenate([[0], np.cumsum(counts)])[:-1]
    rank = np.arange(N_EDGES, dtype=np.int64) - starts[sorted_r]
    l_of = sorted_r % RPC
    core_of = sorted_r // RPC
    dst_slot = prefix[l_of] * EBLK + rank

    # swizzled edge layout: [core, chunk, p, blk_in_chunk, feat] so each chunk's
    # DMA is a fully contiguous [128, CHUNK_BLKS*128] 2D slice per partition
    blk_of = dst_slot // EBLK
    ea_all = np.zeros((NCORES, nchunk, EBLK, CHUNK_BLKS, D), dtype=bf16)
    ea_all[core_of, blk_of // CHUNK_BLKS, dst_slot % EBLK, blk_of % CHUNK_BLKS] = (
        edge_attr[order].astype(bf16)
    )
    ea_all = ea_all.reshape(NCORES, nchunk * EBLK, CHUNK_BLKS * D)

    colr = np.full((NCORES, s_alloc), -1, dtype=np.int32)
    colr[core_of, dst_slot] = (col[order] % RW).astype(np.int32)

    # rebased col per edge slot for DVE-built one-hots:
    # colrT[c, p, blk] = col of edge slot blk*128+p (f32; is_equal needs f32)
    colrT_all = np.ascontiguousarray(
        colr.astype(np.float32).reshape(NCORES, nblk_alloc, EBLK).transpose(0, 2, 1)
    )

    # host-built fp8 one-hots for an evenly-spread subset of blocks in
    # EVERY chunk (keeps DVE and the oh DMA stream co-scheduled)
    flags = _host_blk_flags()
    hcos = np.flatnonzero(flags)
    k = len(hcos)
    one_fp8 = fp8(1.0).view(np.uint8)
    colr_h = colr.reshape(NCORES, nchunk, CHUNK_BLKS, EBLK)[:, :, hcos]
    oh_bits = (colr_h[:, :, :, :, None] == np.arange(RW, dtype=np.int32)
               ).astype(np.uint8) * one_fp8     # [c, chunk, k, EBLK, RW]
    oh_all = (
        oh_bits.view(fp8)
        .transpose(0, 1, 3, 2, 4)               # [c, chunk, p, kblk, n]
        .reshape(NCORES, nchunk * EBLK, k * RW)
    )
    oh_all = np.ascontiguousarray(oh_all)

    x_pad = np.zeros((N_PAD, D), dtype=np.float32)
    x_pad[:N_NODES] = x
    xT_all = np.ascontiguousarray(
        x_pad.reshape(NCORES, NPC, D).transpose(0, 2, 1)
    ).astype(bf16)

    # natural-layout x + b2 in per-group tile layout [NGRP*128, 512]:
    # tile[g*128 + p, nt*128 + d] = (x + b2)[g*512 + nt*128 + p, d]
    xbn = (x_pad + b2[None, :]).reshape(NCORES, NPC, D)
    pad_cols = NGRP * NB_MLP - NPC                   # 256
    xbn = np.concatenate(
        [xbn, np.zeros((NCORES, pad_cols, D), np.float32)], axis=1
    ).reshape(NCORES, NGRP, NB_MLP // D, D, D)       # [c, g, nt, p, d]
    xbn_all = np.ascontiguousarray(
        xbn.transpose(0, 1, 3, 2, 4).reshape(NCORES, NGRP * D, NB_MLP)
    ).astype(bf16)

    batch_pad = np.concatenate(
        [batch, np.full(N_PAD - N_NODES, batch[-1], dtype=batch.dtype)]
    ).astype(np.int64)
    ug = u[batch_pad]                                # [N_PAD, DG]
    ugT_all = np.ascontiguousarray(
        ug.reshape(NCORES, NPC, DG).transpose(0, 2, 1)
    ).astype(bf16)

    consts = {
        "w1x": np.ascontiguousarray(W1[:D]).astype(bf16),     # [128, 256]
        "w1a": np.ascontiguousarray(W1[D : 2 * D]).astype(bf16),
        "w1u": np.ascontiguousarray(W1[2 * D :]).astype(bf16),  # [16, 256]
        "b1t": np.ascontiguousarray(b1.reshape(2, D).T),      # [128, 2] f32
        "w2a": np.ascontiguousarray(W2[:D]).astype(bf16),     # [128, 128]
        "w2b": np.ascontiguousarray(W2[D:]).astype(bf16),     # [128, 128]
        "ident": np.eye(D, dtype=np.float32).astype(bf16),
        "iota": np.tile(np.arange(RW, dtype=np.float32), (EBLK, 1)).astype(bf16),
    }

    in_maps = []
    for c in range(NCORES):
        m = {
            "ea": ea_all[c],
            "oh": oh_all[c],
            "colrt": colrT_all[c],
            "xt": xT_all[c],
            "xbn": xbn_all[c],
            "ugt": ugT_all[c],
        }
        m.update(consts)
        in_maps.append(m)
    return in_maps, B, nchunk, nblk_alloc, k


def _build_program(B, nchunk, nblk_alloc, k_host):
    import concourse.bacc as bacc
    import concourse.mybir as mybir
    import concourse.tile as tile

    F32 = mybir.dt.float32
    BF16 = mybir.dt.bfloat16
    FP8 = mybir.dt.float8e4
    prefix = np.concatenate([[0], np.cumsum(B)])
    flags = _host_blk_flags()
    hidx = np.cumsum([0] + flags[:-1])      # blk-in-chunk -> host-slot index

    nc = bacc.Bacc("TRN2", target_bir_lowering=False, debug=False)

    ea_d = nc.dram_tensor("ea", [nchunk * EBLK, CHUNK_BLKS * D], BF16,
                          kind="ExternalInput")
    oh_d = nc.dram_tensor("oh", [nchunk * EBLK, k_host * RW], FP8,
                          kind="ExternalInput")
    colrt_d = nc.dram_tensor("colrt", [EBLK, nblk_alloc], F32,
                             kind="ExternalInput")
    xt_d = nc.dram_tensor("xt", [D, NPC], BF16, kind="ExternalInput")
    xbn_d = nc.dram_tensor("xbn", [NGRP * D, NB_MLP], BF16, kind="ExternalInput")
    ugt_d = nc.dram_tensor("ugt", [DG, NPC], BF16, kind="ExternalInput")
    w1x_d = nc.dram_tensor("w1x", [D, H], BF16, kind="ExternalInput")
    w1a_d = nc.dram_tensor("w1a", [D, H], BF16, kind="ExternalInput")
    w1u_d = nc.dram_tensor("w1u", [DG, H], BF16, kind="ExternalInput")
    b1t_d = nc.dram_tensor("b1t", [D, 2], F32, kind="ExternalInput")
    w2a_d = nc.dram_tensor("w2a", [D, D], BF16, kind="ExternalInput")
    w2b_d = nc.dram_tensor("w2b", [D, D], BF16, kind="ExternalInput")
    ident_d = nc.dram_tensor("ident", [D, D], BF16, kind="ExternalInput")
    iota_d = nc.dram_tensor("iota", [EBLK, RW], BF16, kind="ExternalInput")
    out_d = nc.dram_tensor("out", [NGRP * D, NB_MLP], BF16,
                           kind="ExternalOutput")

    with tile.TileContext(nc) as tc, ExitStack() as ctx:
        persist = ctx.enter_context(tc.tile_pool(name="persist", bufs=1))
        ea_pool = ctx.enter_context(tc.tile_pool(name="ea", bufs=6))
        ohc_pool = ctx.enter_context(tc.tile_pool(name="ohc", bufs=3))
        ohv_pool = ctx.enter_context(tc.tile_pool(name="ohv", bufs=24))
        agg_pool = ctx.enter_context(tc.tile_pool(name="agg", bufs=8))
        ug_pool = ctx.enter_context(tc.tile_pool(name="ug", bufs=2))
        xb_pool = ctx.enter_context(tc.tile_pool(name="xb", bufs=2))
        hs_pool = ctx.enter_context(tc.tile_pool(name="hs", bufs=4))
        os_pool = ctx.enter_context(tc.tile_pool(name="os", bufs=2))
        sc_psum = ctx.enter_context(tc.tile_pool(name="scps", bufs=4, space="PSUM"))
        h_psum = ctx.enter_context(tc.tile_pool(name="hps", bufs=2, space="PSUM"))
        o2_psum = ctx.enter_context(tc.tile_pool(name="o2ps", bufs=2, space="PSUM"))

        # --- persistent loads -------------------------------------------------
        def pload(dram, shape, dtype, engine):
            t = persist.tile(shape, dtype, tag=dram.name)
            engine.dma_start(t[:], dram.ap())
            return t

        # order matters: the first scatter blocks only need iota + colrt,
        # so issue those first; xt (3.2 MB) is not needed until mlp_group(0)
        iota_t = pload(iota_d, [EBLK, RW], BF16, nc.scalar)
        colrt_t = pload(colrt_d, [EBLK, nblk_alloc], F32, nc.scalar)
        w1x_t = pload(w1x_d, [D, H], BF16, nc.scalar)
        w1a_t = pload(w1a_d, [D, H], BF16, nc.scalar)
        w1u_t = pload(w1u_d, [DG, H], BF16, nc.scalar)
        b1t_t = pload(b1t_d, [D, 2], F32, nc.scalar)
        w2a_t = pload(w2a_d, [D, D], BF16, nc.scalar)
        w2b_t = pload(w2b_d, [D, D], BF16, nc.scalar)
        ident_t = pload(ident_d, [D, D], BF16, nc.scalar)
        xt_t = pload(xt_d, [D, NPC], BF16, nc.scalar)

        chunk_tiles = {}

        def get_chunk(ci):
            if ci not in chunk_tiles:
                ea_t = ea_pool.tile([EBLK, CHUNK_BLKS * D], BF16, tag="each")
                nc.sync.dma_start(
                    ea_t[:], ea_d.ap()[ci * EBLK : (ci + 1) * EBLK, :]
                )
                oh_t = ohc_pool.tile([EBLK, k_host * RW], FP8, tag="ohch")
                nc.gpsimd.dma_start(
                    oh_t[:], oh_d.ap()[ci * EBLK : (ci + 1) * EBLK, :]
                )
                chunk_tiles[ci] = (ea_t, oh_t)
            return chunk_tiles[ci]

        agg_tiles = [None] * (RPC // 2 + 1)

        def scatter_range(l):
            ps = sc_psum.tile([D, RW], F32, tag="scps")
            nb = int(B[l])
            for b in range(nb):
                blk = int(prefix[l]) + b
                ea_t, oh_t = get_chunk(blk // CHUNK_BLKS)
                co = blk % CHUNK_BLKS
                if flags[co]:
                    hi = int(hidx[co])
                    rhs = oh_t[:, hi * RW : (hi + 1) * RW]
                else:
                    ohv = ohv_pool.tile([EBLK, RW], BF16, tag="ohv")
                    nc.vector.tensor_scalar(
                        ohv[:], iota_t[:], colrt_t[:, blk : blk + 1], None,
                        mybir.AluOpType.is_equal,
                    )
                    rhs = ohv[:]
                nc.tensor.matmul(
                    ps[:],
                    ea_t[:, co * D : (co + 1) * D],
                    rhs,
                    start=(b == 0),
                    stop=(b == nb - 1),
                )
            # pack two 128-node ranges into one [128, 256] agg tile so the
            # MLP agg-term matmul keeps N=256
            if l % 2 == 0:
                agg_tiles[l // 2] = agg_pool.tile([D, 2 * RW], BF16, tag="agg",
                                                  name="aggp")
            at = agg_tiles[l // 2]
            nc.scalar.copy(at[:, (l % 2) * RW : (l % 2 + 1) * RW], ps[:])

        Relu = mybir.ActivationFunctionType.Relu

        def mlp_group(g):
            gs = g * NB_MLP
            nb = min(NB_MLP, NPC - gs)
            pairs = [j for j in (2 * g, 2 * g + 1) if j * 2 * RW < gs + nb]
            ug_t = ug_pool.tile([DG, nb], BF16, tag="ug")
            nc.scalar.dma_start(ug_t[:], ugt_d.ap()[:, gs : gs + nb])
            xb_t = xb_pool.tile([D, NB_MLP], BF16, tag="xb")
            nc.scalar.dma_start(xb_t[:], xbn_d.ap()[g * D : (g + 1) * D, :])
            hs = []
            for ht in range(2):
                hp = h_psum.tile([D, nb], F32, tag="hps")
                hsl = slice(ht * D, (ht + 1) * D)
                nc.tensor.matmul(
                    hp[:], w1x_t[:, hsl], xt_t[:, gs : gs + nb], start=True, stop=False
                )
                for j in pairs:
                    o0 = j * 2 * RW - gs
                    nc.tensor.matmul(
                        hp[:, o0 : o0 + 2 * RW],
                        w1a_t[:, hsl],
                        agg_tiles[j][:],
                        start=False,
                        stop=False,
                    )
                nc.tensor.matmul(
                    hp[:], w1u_t[:, hsl], ug_t[:], start=False, stop=True
                )
                ht_sb = hs_pool.tile([D, nb], BF16, tag="hs")
                nc.scalar.activation(
                    ht_sb[:], hp[:], Relu, bias=b1t_t[:, ht : ht + 1]
                )
                hs.append(ht_sb)
            # layer 2 per 128-node tile in natural orientation [n, d]
            o_sb = os_pool.tile([D, nb], BF16, tag="os")
            for nt in range(nb // D):
                nsl = slice(nt * D, (nt + 1) * D)
                o2 = o2_psum.tile([D, D], F32, tag="o2ps")
                nc.tensor.matmul(o2[:], hs[0][:, nsl], w2a_t[:],
                                 start=True, stop=False)
                nc.tensor.matmul(o2[:], hs[1][:, nsl], w2b_t[:],
                                 start=False, stop=False)
                # residual + bias: += I.T @ (x + b2) tile
                nc.tensor.matmul(o2[:], ident_t[:], xb_t[:, nsl],
                                 start=False, stop=True)
                nc.scalar.copy(o_sb[:, nsl], o2[:])
            nc.sync.dma_start(out_d.ap()[g * D : (g + 1) * D, :nb], o_sb[:])

        for g in range(NGRP):
            for l in (4 * g, 4 * g + 1, 4 * g + 2, 4 * g + 3):
                if l < RPC:
                    scatter_range(l)
            mlp_group(g)

    nc.compile()
    return nc


def kernel(**inputs) -> np.ndarray:
    in_maps, B, nchunk, nblk_alloc, nhost = _shard_inputs(
        inputs["x"], inputs["edge_index"], inputs["edge_attr"], inputs["u"],
        inputs["batch"], inputs["W1"], inputs["b1"], inputs["W2"], inputs["b2"],
    )
    nc = _build_program(B, nchunk, nblk_alloc, nhost)

    from concourse.bass_utils import run_bass_kernel_spmd

    want_trace = bool(os.environ.get("KPROF"))
    if want_trace:
        try:
            from antenv.axon_hooks import get_axon_ntff_profile_hook  # noqa: F401
        except ImportError:
            want_trace = False
    res = run_bass_kernel_spmd(nc, in_maps, list(range(NCORES)), trace=want_trace)
    _PROFILE_RESULTS[0] = res
    # un-swizzle the per-group tile layout back to [N, D] and upcast
    out = np.empty((NCORES, NPC, D), dtype=np.float32)
    for c in range(NCORES):
        t = res.results[c]["out"].astype(np.float32)     # [NGRP*128, 512]
        t = t.reshape(NGRP, D, NB_MLP // D, D).transpose(0, 2, 1, 3)
        out[c] = t.reshape(NGRP * NB_MLP, D)[:NPC]
    out = out.reshape(N_PAD, D)
    return np.ascontiguousarray(out[:N_NODES])
